# revision 12
# baseline (speedup 1.0000x reference)
"""CurvatureEnhancedGAT — 3-layer GAT on Trainium2, 8-core SPMD.

Strategy (graph/data parallel, dst-sharded):
- Nodes are range-partitioned across 8 cores (6250 each). Edges (incl.
  self-loops) are sorted by dst on the host, bucketed per (core, 128-dst
  tile), re-sorted by src within each tile and split at src=32768 (int16
  index limit of the gpsimd dma_gather ucode), then padded to uniform
  per-tile chunk counts so one compiled program serves all cores.
- Gather table row (512B): [h bf16 x128 | as f32 | ad f32 | pad]. A second
  local table AD_LOC (256B rows) provides ad[dst] by local dst index.
- Per layer:
  Phase A (own nodes): one matmul per 128-node tile computes
    [h | as | ad] = x_tile @ [W | W@Asrc | W@Adst] node-major (lhsT is the
    feature-major activation tile). Stage + DMA to DRAM, AllGather.
  Phase B (own dst tiles): 3 dma_gathers fetch h/as rows (low+high src
    ranges) and ad[dst] rows for all edges of the tile. Attention:
    e = leakyrelu(as_src + ad_dst), ex = exp(e) (no max subtraction —
    bounded, validated vs reference). Per 128-edge chunk a one-hot
    S[e, dst_local] (tensor_scalar is_equal vs iota row) and one matmul
    accumulate [ex*h | ex] into PSUM — softmax numerator+denominator in one
    accumulation group. out = num/den (+bias), relu(scale*x) between
    layers; own output is PE-transposed back to feature-major for the next
    layer's phase A. Only collective: 1 AllGather per layer.

kernel(**inputs) takes FULL inputs, returns the FULL output.
"""

import os
from contextlib import ExitStack

import numpy as np
import ml_dtypes

import concourse.bass as bass
import concourse.bacc as bacc
import concourse.mybir as mybir
import concourse.tile as tile

P = 128
M = 8  # cores

F32 = mybir.dt.float32
BF16 = mybir.dt.bfloat16
I16 = mybir.dt.int16

ROW = 256      # bf16 elements per gather-table row (512B)
ADROW = 64     # f32 elements per AD_LOC row (256B)
SPLIT = 32768  # int16 index split

NEG_SLOPE = 0.2
EPS = 1e-16

LAST_RESULTS = None


def build_gat(NPC, NL, NH, scale, layers, use_bias):
    """NL/NH: per-tile low/high chunk counts (lists, uniform across cores)."""
    N = M * NPC
    TILES = -(-NPC // P)
    LAST = NPC - (TILES - 1) * P
    FL = layers[-1]["F"]
    NCH = [NL[t] + NH[t] for t in range(TILES)]
    CO_LO = np.concatenate([[0], np.cumsum(NL)]).astype(int)    # in chunks
    CO_HI = np.concatenate([[0], np.cumsum(NH)]).astype(int)
    CO_CH = np.concatenate([[0], np.cumsum(NCH)]).astype(int)
    SUM_LO, SUM_HI, SUM_CH = int(CO_LO[-1]), int(CO_HI[-1]), int(CO_CH[-1])

    nc = bacc.Bacc("TRN2", num_devices=M)

    # ---- kernel I/O ----
    xT_d = nc.dram_tensor("xT", [P, NPC], F32, kind="ExternalInput")
    wext_d = []
    for li, lp in enumerate(layers):
        CW = lp["F"] + 2 * lp["H"]
        wext_d.append(nc.dram_tensor(f"wext{li}", [P, CW], F32, kind="ExternalInput"))
    bias_d = []
    for li, lp in enumerate(layers):
        bias_d.append(
            nc.dram_tensor(f"bias{li}", [P, lp["F"]], F32, kind="ExternalInput")
            if use_bias[li] else None
        )
    iota_d = nc.dram_tensor("iota", [P, P], F32, kind="ExternalInput")
    ident_d = nc.dram_tensor("ident", [P, P], F32, kind="ExternalInput")
    ilo_d = nc.dram_tensor("ilo", [P, max(8 * SUM_LO, 8)], I16, kind="ExternalInput")
    ihi_d = nc.dram_tensor("ihi", [P, max(8 * SUM_HI, 8)], I16, kind="ExternalInput")
    iad_d = nc.dram_tensor("iad", [P, 8 * SUM_CH], I16, kind="ExternalInput")
    edloc_d = nc.dram_tensor("edloc", [P, SUM_CH], F32, kind="ExternalInput")
    out_d = nc.dram_tensor("out", [NPC, FL], F32, kind="ExternalOutput")

    with ExitStack() as ctx:
        tc = ctx.enter_context(tile.TileContext(nc))
        pers = ctx.enter_context(tc.tile_pool(name="pers", bufs=1))
        stage_p = ctx.enter_context(tc.tile_pool(name="stage", bufs=1))
        dram_in = ctx.enter_context(tc.tile_pool(name="dram_in", bufs=2, space="DRAM"))
        dram_ad = ctx.enter_context(tc.tile_pool(name="dram_ad", bufs=2, space="DRAM"))
        dram_sh = ctx.enter_context(tc.tile_pool(name="dram_sh", bufs=2, space="DRAM"))
        hg_p = ctx.enter_context(tc.tile_pool(name="hg", bufs=2))
        ad_p = ctx.enter_context(tc.tile_pool(name="ad", bufs=2))
        v_p = ctx.enter_context(tc.tile_pool(name="v", bufs=2))
        s_p = ctx.enter_context(tc.tile_pool(name="s", bufs=4))
        sm_p = ctx.enter_context(tc.tile_pool(name="sm", bufs=3))
        o_p = ctx.enter_context(tc.tile_pool(name="o", bufs=2))
        psA_p = ctx.enter_context(tc.tile_pool(name="psA", bufs=2, space="PSUM"))
        psB_p = ctx.enter_context(tc.tile_pool(name="psB", bufs=2, space="PSUM"))
        psT_p = ctx.enter_context(tc.tile_pool(name="psT", bufs=2, space="PSUM"))

        # ---- persistent SBUF ----
        xt_a = pers.tile([P, NPC], F32, name="xt_a")
        xt_b = pers.tile([P, NPC], F32, name="xt_b")
        iota_sb = pers.tile([P, P], F32, name="iota_sb")
        ident_sb = pers.tile([P, P], F32, name="ident_sb")
        ilo_sb = pers.tile([P, max(8 * SUM_LO, 8)], I16, name="ilo_sb")
        ihi_sb = pers.tile([P, max(8 * SUM_HI, 8)], I16, name="ihi_sb")
        iad_sb = pers.tile([P, 8 * SUM_CH], I16, name="iad_sb")
        edloc_sb = pers.tile([P, SUM_CH], F32, name="edloc_sb")
        wext_sb = []
        bias_sb = []
        for li, lp in enumerate(layers):
            CW = lp["F"] + 2 * lp["H"]
            t = pers.tile([P, CW], F32, name=f"wext_sb{li}")
            nc.sync.dma_start(out=t[:], in_=wext_d[li][:])
            wext_sb.append(t)
            if bias_d[li] is not None:
                bt = pers.tile([P, lp["F"]], F32, name=f"bias_sb{li}")
                nc.sync.dma_start(out=bt[:], in_=bias_d[li][:])
                bias_sb.append(bt)
            else:
                bias_sb.append(None)

        nc.sync.dma_start(out=xt_a[:], in_=xT_d[:])
        nc.sync.dma_start(out=iota_sb[:], in_=iota_d[:])
        nc.sync.dma_start(out=ident_sb[:], in_=ident_d[:])
        nc.sync.dma_start(out=ilo_sb[:], in_=ilo_d[:])
        nc.sync.dma_start(out=ihi_sb[:], in_=ihi_d[:])
        nc.sync.dma_start(out=iad_sb[:], in_=iad_d[:])
        nc.sync.dma_start(out=edloc_sb[:], in_=edloc_d[:])

        # staging tiles (layer-independent shapes); zero once so the pad
        # columns the table DMAs read are initialized
        hstage = pers.tile([P, TILES * ROW], I16, name="hstage")
        adstage = pers.tile([P, TILES * ADROW], F32, name="adstage")
        nc.gpsimd.memset(hstage[:], 0)
        nc.gpsimd.memset(adstage[:], 0)

        xt_cur, xt_nxt = xt_a, xt_b
        n_layers = len(layers)

        for li, lp in enumerate(layers):
            F, H = lp["F"], lp["H"]
            C = F // H
            CW = F + 2 * H
            EW = F + H
            last_layer = li == n_layers - 1

            # ---------- phase A ----------
            hstage_bf = hstage[:].bitcast(BF16)  # [128, TILES*ROW]
            hstage_f = hstage[:].bitcast(F32)    # [128, TILES*ROW//2]
            for t in range(TILES):
                rows = LAST if t == TILES - 1 else P
                pa = psA_p.tile([P, CW], F32, name="pa", tag="pa")
                nc.tensor.matmul(
                    out=pa[:rows, :],
                    lhsT=xt_cur[:, t * P : t * P + rows],
                    rhs=wext_sb[li][:],
                    start=True,
                    stop=True,
                )
                # h -> bf16 cols [t*ROW, t*ROW+F)
                nc.vector.tensor_copy(
                    out=hstage_bf[:rows, t * ROW : t * ROW + F], in_=pa[:rows, 0:F]
                )
                # as|ad -> f32 cols [t*128+64, +2H)
                nc.vector.tensor_copy(
                    out=hstage_f[:rows, t * (ROW // 2) + F // 2 :
                                 t * (ROW // 2) + F // 2 + 2 * H],
                    in_=pa[:rows, F : F + 2 * H],
                )
                # ad -> adstage f32 cols [t*ADROW, +H)
                nc.vector.tensor_copy(
                    out=adstage[:rows, t * ADROW : t * ADROW + H],
                    in_=pa[:rows, F + H : F + 2 * H],
                )

            hext_own = dram_in.tile([NPC, ROW], I16, name="hext_own",
                                    tag="hext_own")
            adloc = dram_ad.tile([NPC, ADROW], F32, name="adloc", tag="adloc")
            if TILES > 1:
                nc.sync.dma_start(
                    out=hext_own[: (TILES - 1) * P, :].rearrange(
                        "(t p) w -> p t w", p=P),
                    in_=hstage[:, : (TILES - 1) * ROW].rearrange(
                        "p (t w) -> p t w", w=ROW),
                )
                nc.sync.dma_start(
                    out=adloc[: (TILES - 1) * P, :].rearrange("(t p) w -> p t w", p=P),
                    in_=adstage[:, : (TILES - 1) * ADROW].rearrange(
                        "p (t w) -> p t w", w=ADROW),
                )
            nc.sync.dma_start(
                out=hext_own[(TILES - 1) * P :, :],
                in_=hstage[:LAST, (TILES - 1) * ROW : TILES * ROW],
            )
            nc.sync.dma_start(
                out=adloc[(TILES - 1) * P :, :],
                in_=adstage[:LAST, (TILES - 1) * ADROW : TILES * ADROW],
            )

            hext_full = dram_sh.tile([N, ROW], I16, name="hext_full",
                                     tag="hext_full", addr_space="Shared")
            nc.gpsimd.collective_compute(
                "AllGather",
                mybir.AluOpType.bypass,
                replica_groups=[list(range(M))],
                ins=[hext_own[:].opt()],
                outs=[hext_full[:].opt()],
            )

            # ---------- phase B ----------
            ostage = None
            if last_layer:
                ostage = stage_p.tile([P, TILES * F], F32, name="ostage",
                                      tag="ostage")

            for t in range(TILES):
                rows = LAST if t == TILES - 1 else P
                nch = NCH[t]
                nl, nh = NL[t], NH[t]

                hg = hg_p.tile([P, nch * ROW], I16, name="hg", tag="hg")

                def gather_batched(out_tile, ocol0, table, idx_sb, icol0, nchunks,
                                   elem, GMAX=6):
                    done = 0
                    while done < nchunks:
                        g = min(GMAX, nchunks - done)
                        nc.gpsimd.dma_gather(
                            out_ap=out_tile[:, ocol0 + done * elem :
                                            ocol0 + (done + g) * elem].rearrange(
                                "p (n w) -> p n w", w=elem),
                            in_ap=table,
                            idxs_ap=idx_sb[:, 8 * (icol0 + done) :
                                           8 * (icol0 + done + g)],
                            num_idxs=g * P,
                            num_idxs_reg=g * P,
                            elem_size=elem,
                        )
                        done += g

                if nl:
                    gather_batched(hg, 0, hext_full[:], ilo_sb, CO_LO[t], nl, ROW)
                if nh:
                    gather_batched(hg, nl * ROW, hext_full[SPLIT:, :], ihi_sb,
                                   CO_HI[t], nh, ROW)
                adg = ad_p.tile([P, nch * ADROW], F32, name="adg", tag="adg")
                gather_batched(adg, 0, adloc[:], iad_sb, CO_CH[t], nch, ADROW)

                hg_f = hg[:].bitcast(F32)  # [128, nch*128]
                hg3f = hg_f.rearrange("p (n w) -> p n w", w=ROW // 2)
                hg3b = hg[:].bitcast(BF16).rearrange("p (n w) -> p n w", w=ROW)
                ad3 = adg[:].rearrange("p (n w) -> p n w", w=ADROW)

                # e = as_src + ad_dst  [128, nch, H] f32
                e_pre = sm_p.tile([P, nch * H], F32, name="e_pre", tag="e_pre")
                nc.vector.tensor_tensor(
                    out=e_pre[:].rearrange("p (n h) -> p n h", h=H),
                    in0=hg3f[:, :, F // 2 : F // 2 + H],
                    in1=ad3[:, :, 0:H],
                    op=mybir.AluOpType.add,
                )
                # leaky relu
                e_lr = sm_p.tile([P, nch * H], F32, name="e_lr", tag="e_lr")
                nc.vector.scalar_tensor_tensor(
                    out=e_lr[:], in0=e_pre[:], scalar=NEG_SLOPE, in1=e_pre[:],
                    op0=mybir.AluOpType.mult, op1=mybir.AluOpType.max,
                )
                # ex = exp(e)
                ex_f = sm_p.tile([P, nch * H], F32, name="ex_f", tag="ex_f")
                nc.scalar.activation(
                    out=ex_f[:], in_=e_lr[:],
                    func=mybir.ActivationFunctionType.Exp,
                )
                # V = [ex*h | ex] bf16
                V = v_p.tile([P, nch * EW], BF16, name="V", tag="V")
                v3 = V[:].rearrange("p (n w) -> p n w", w=EW)
                nc.vector.tensor_copy(
                    out=v3[:, :, F : F + H],
                    in_=ex_f[:].rearrange("p (n h) -> p n h", h=H),
                )
                nc.vector.tensor_tensor(
                    out=v3[:, :, 0:F].rearrange("p n (h c) -> p n h c", c=C),
                    in0=hg3b[:, :, 0:F].rearrange("p n (h c) -> p n h c", c=C),
                    in1=v3[:, :, F : F + H].unsqueeze(3).broadcast_to(
                        [P, nch, H, C]),
                    op=mybir.AluOpType.mult,
                )

                # accumulate [num | den] over chunks
                pb = psB_p.tile([P, EW], F32, name="pb", tag="pb")
                for j in range(nch):
                    S = s_p.tile([P, P], BF16, name="S", tag="S")
                    nc.vector.tensor_scalar(
                        out=S[:], in0=iota_sb[:],
                        scalar1=edloc_sb[:, CO_CH[t] + j : CO_CH[t] + j + 1],
                        scalar2=None, op0=mybir.AluOpType.is_equal,
                    )
                    nc.tensor.matmul(
                        out=pb[:], lhsT=S[:], rhs=V[:, j * EW : (j + 1) * EW],
                        start=(j == 0), stop=(j == nch - 1),
                    )

                # out = num / (den + eps)
                den = sm_p.tile([P, H], F32, name="den", tag="den")
                nc.vector.tensor_scalar(
                    out=den[:], in0=pb[:, F : F + H], scalar1=EPS, scalar2=None,
                    op0=mybir.AluOpType.add,
                )
                recip = sm_p.tile([P, H], F32, name="recip", tag="recip")
                nc.vector.reciprocal(recip[:], den[:])

                if not last_layer:
                    outt = o_p.tile([P, F], F32, name="outt", tag="outt")
                    nc.vector.tensor_tensor(
                        out=outt[:].rearrange("p (h c) -> p h c", c=C),
                        in0=pb[:, 0:F].rearrange("p (h c) -> p h c", c=C),
                        in1=recip[:].unsqueeze(2).broadcast_to([P, H, C]),
                        op=mybir.AluOpType.mult,
                    )
                    if bias_sb[li] is not None:
                        nc.vector.tensor_tensor(
                            out=outt[:], in0=outt[:], in1=bias_sb[li][:],
                            op=mybir.AluOpType.add,
                        )
                    act = o_p.tile([P, F], F32, name="act", tag="act")
                    nc.vector.tensor_scalar(
                        out=act[:], in0=outt[:], scalar1=float(scale), scalar2=0.0,
                        op0=mybir.AluOpType.mult, op1=mybir.AluOpType.max,
                    )
                    pt = psT_p.tile([P, P], F32, name="pt", tag="pt")
                    nc.tensor.transpose(
                        out=pt[:, :rows], in_=act[:rows, :],
                        identity=ident_sb[:rows, :rows],
                    )
                    nc.vector.tensor_copy(
                        out=xt_nxt[:, t * P : t * P + rows], in_=pt[:, :rows]
                    )
                else:
                    oseg = ostage[:, t * F : (t + 1) * F]
                    nc.vector.tensor_tensor(
                        out=oseg.rearrange("p (h c) -> p h c", c=C),
                        in0=pb[:, 0:F].rearrange("p (h c) -> p h c", c=C),
                        in1=recip[:].unsqueeze(2).broadcast_to([P, H, C]),
                        op=mybir.AluOpType.mult,
                    )
                    if bias_sb[li] is not None:
                        nc.vector.tensor_tensor(
                            out=oseg, in0=oseg, in1=bias_sb[li][:],
                            op=mybir.AluOpType.add,
                        )

            if last_layer:
                if TILES > 1:
                    nc.sync.dma_start(
                        out=out_d[: (TILES - 1) * P, :].rearrange(
                            "(t p) f -> p t f", p=P),
                        in_=ostage[:, : (TILES - 1) * F].rearrange(
                            "p (t f) -> p t f", f=F),
                    )
                nc.sync.dma_start(
                    out=out_d[(TILES - 1) * P :, :],
                    in_=ostage[:LAST, (TILES - 1) * F : TILES * F],
                )
            else:
                xt_cur, xt_nxt = xt_nxt, xt_cur

    nc.finalize()
    return nc


# ---------------------------------------------------------------------------
# host-side preprocessing
# ---------------------------------------------------------------------------

def _a_mat(a):
    H, C = a.shape
    A = np.zeros((H * C, H), np.float32)
    for h in range(H):
        A[h * C : (h + 1) * C, h] = a[h]
    return A


def _wrap16(idx):
    """[n] int -> [128, n//16] int16, wrapped in 16 partitions, replicated."""
    n = len(idx)
    assert n % 16 == 0
    w = np.asarray(idx, np.int16).reshape(n // 16, 16).T  # [16, cols]
    return np.tile(w, (8, 1))


def prepare_host(x, edge_index, curvature_weights, weights, NPC):
    N = M * NPC
    TILES = -(-NPC // P)

    src = np.concatenate([np.asarray(edge_index[0]), np.arange(N)]).astype(np.int64)
    dst = np.concatenate([np.asarray(edge_index[1]), np.arange(N)]).astype(np.int64)
    order = np.argsort(dst, kind="stable")
    src_s = src[order].astype(np.int64)
    dst_s = dst[order].astype(np.int64)

    scale = 1.0 + 0.3 * float(np.mean(np.asarray(curvature_weights)))

    tile_starts = np.array([c * NPC + t * P for c in range(M) for t in range(TILES)])
    tile_ends = np.array(
        [min(c * NPC + (t + 1) * P, (c + 1) * NPC) for c in range(M)
         for t in range(TILES)])
    lo = np.searchsorted(dst_s, tile_starts, side="left")
    hi = np.searchsorted(dst_s, tile_ends, side="left")

    # per (core, tile): sort by src, split at SPLIT, count chunks
    per = {}
    nlo_ct = np.zeros((M, TILES), int)
    nhi_ct = np.zeros((M, TILES), int)
    for c in range(M):
        for t in range(TILES):
            i = c * TILES + t
            a, b = int(lo[i]), int(hi[i])
            s = src_s[a:b]
            d = dst_s[a:b]
            o2 = np.argsort(s, kind="stable")
            s, d = s[o2], d[o2]
            k = int(np.searchsorted(s, SPLIT, side="left"))
            per[(c, t)] = (s, d, k)
            nlo_ct[c, t] = -(-k // P)
            nhi_ct[c, t] = -(-(len(s) - k) // P)

    NL = [int(nlo_ct[:, t].max()) for t in range(TILES)]
    NH = [int(nhi_ct[:, t].max()) for t in range(TILES)]
    NCH = [NL[t] + NH[t] for t in range(TILES)]
    SUM_LO, SUM_HI, SUM_CH = sum(NL), sum(NH), sum(NCH)
    CO_LO = np.concatenate([[0], np.cumsum(NL)]).astype(int)
    CO_HI = np.concatenate([[0], np.cumsum(NH)]).astype(int)
    CO_CH = np.concatenate([[0], np.cumsum(NCH)]).astype(int)

    ilo = np.zeros((M, P, max(8 * SUM_LO, 8)), np.int16)
    ihi = np.zeros((M, P, max(8 * SUM_HI, 8)), np.int16)
    iad = np.zeros((M, P, 8 * SUM_CH), np.int16)
    edloc = np.full((M, P, SUM_CH), -1.0, np.float32)

    for c in range(M):
        for t in range(TILES):
            s, d, k = per[(c, t)]
            base = tile_starts[c * TILES + t]
            nl, nh, nch = NL[t], NH[t], NCH[t]
            # padded per-class edge lists (pad src-idx 0, dloc -1, dst-local 0)
            s_lo = np.zeros(nl * P, np.int64); s_lo[:k] = s[:k]
            s_hi = np.zeros(nh * P, np.int64); s_hi[: len(s) - k] = s[k:] - SPLIT
            dl = np.full(nch * P, -1.0, np.float32)
            dmix = np.zeros(nch * P, np.int64)
            dl[:k] = (d[:k] - base).astype(np.float32)
            dmix[:k] = d[:k] - c * NPC
            off = nl * P
            dl[off : off + len(s) - k] = (d[k:] - base).astype(np.float32)
            dmix[off : off + len(s) - k] = d[k:] - c * NPC
            if nl:
                ilo[c, :, 8 * CO_LO[t] : 8 * CO_LO[t + 1]] = _wrap16(s_lo)
            if nh:
                ihi[c, :, 8 * CO_HI[t] : 8 * CO_HI[t + 1]] = _wrap16(s_hi)
            iad[c, :, 8 * CO_CH[t] : 8 * CO_CH[t + 1]] = _wrap16(dmix)
            # chunk-slot layout for edloc: slot p of chunk j = flat j*128+p
            edloc[c, :, CO_CH[t] : CO_CH[t + 1]] = dl.reshape(nch, P).T

    layers = []
    wext = []
    biases = []
    use_bias = []
    for (W, a_s, a_d, b) in weights:
        W = np.asarray(W, np.float32)
        a_s = np.asarray(a_s, np.float32)
        a_d = np.asarray(a_d, np.float32)
        b = np.asarray(b, np.float32)
        H, C = a_s.shape
        layers.append({"F": H * C, "H": H})
        wext.append(np.concatenate(
            [W, W @ _a_mat(a_s), W @ _a_mat(a_d)], axis=1).astype(np.float32))
        ub = bool(np.any(b != 0))
        use_bias.append(ub)
        biases.append(np.tile(b[None, :], (P, 1)).astype(np.float32) if ub else None)

    xT = np.ascontiguousarray(np.asarray(x, np.float32).T)
    iota = np.tile(np.arange(P, dtype=np.float32)[None, :], (P, 1))
    ident = np.eye(P, dtype=np.float32)

    in_maps = []
    for c in range(M):
        im = {
            "xT": np.ascontiguousarray(xT[:, c * NPC : (c + 1) * NPC]),
            "iota": iota,
            "ident": ident,
            "ilo": np.ascontiguousarray(ilo[c]),
            "ihi": np.ascontiguousarray(ihi[c]),
            "iad": np.ascontiguousarray(iad[c]),
            "edloc": np.ascontiguousarray(edloc[c]),
        }
        for li in range(len(layers)):
            im[f"wext{li}"] = wext[li]
            if biases[li] is not None:
                im[f"bias{li}"] = biases[li]
        in_maps.append(im)

    return in_maps, layers, use_bias, NL, NH, scale


def run_gat(x, edge_index, curvature_weights, weights, trace=False):
    from concourse.bass_utils import run_bass_kernel_spmd

    global LAST_RESULTS
    N = np.asarray(x).shape[0]
    assert N % M == 0
    NPC = N // M

    in_maps, layers, use_bias, NL, NH, scale = prepare_host(
        x, edge_index, curvature_weights, weights, NPC)
    nc = build_gat(NPC, NL, NH, scale, layers, use_bias)
    if trace:
        trace = _install_ntff_hook_shim()
    try:
        res = run_bass_kernel_spmd(nc, in_maps, core_ids=list(range(M)),
                                   trace=trace)
    except Exception:
        if not trace:
            raise
        res = run_bass_kernel_spmd(nc, in_maps, core_ids=list(range(M)))
    LAST_RESULTS = res
    return np.concatenate([res.results[c]["out"] for c in range(M)], axis=0)


def _install_ntff_hook_shim():
    """antenv.axon_hooks is absent in this image; synthesize it so
    run_bass_kernel_spmd(trace=True) can capture NTFF profiles."""
    import sys
    import types

    if "antenv.axon_hooks" in sys.modules:
        return True
    try:
        import antenv
        import trn_agent_boot.trn_boot as tb

        mod = types.ModuleType("antenv.axon_hooks")
        mod._hook = None
        mod.set_axon_ntff_profile_hook = lambda h: setattr(mod, "_hook", h)
        mod.get_axon_ntff_profile_hook = lambda: mod._hook
        sys.modules["antenv.axon_hooks"] = mod
        antenv.axon_hooks = mod
        mod.set_axon_ntff_profile_hook(
            tb._ntff_profile_via_ctypes("/opt/axon/libaxon_pjrt.so"))
        return True
    except Exception as e:
        print(f"ntff hook shim failed ({e}); running without trace")
        return False


def kernel(
    x,
    edge_index,
    curvature_weights,
    W1, a_src1, a_dst1, b1,
    W2, a_src2, a_dst2, b2,
    W3, a_src3, a_dst3, b3,
):
    weights = [
        (W1, a_src1, a_dst1, b1),
        (W2, a_src2, a_dst2, b2),
        (W3, a_src3, a_dst3, b3),
    ]
    trace = bool(int(os.environ.get("GAT_TRACE", "0")))
    out = run_gat(x, edge_index, curvature_weights, weights, trace=trace)
    return out.astype(np.float32)
